# revision 23
# baseline (speedup 1.0000x reference)
"""Trainium2 Bass kernel for nn_BCNet: three-way low-rank bilinear net.

reference:
  v_ = relu(v @ Wv.T + bv)            # (B, NV, HK)
  q_ = relu(q @ Wq.T + bq)            # (B, NQ, HK)
  logits = einsum('hk,bvk,bqk->bhvq', h_mat, v_, q_) + h_bias

Sharding: data-parallel over batch, 4 batch items per core (8 cores).
Matmuls in fp16 with fp32 PSUM accumulation (fp16 runs at the same PE
rate as bf16 but has 8x finer mantissa, buying error budget), except
the first 512 of v's 2048-deep contraction, which runs as two
fp8-e4m3 DoubleRow matmuls per 128-wide j-block (2x MAC rate; operands
scale-split v/8 x 8*Wv so the product accumulates unscaled into the
same PSUM group). Measured end-to-end rel err 1.682e-2 vs the 2e-2
gate; a numpy simulation of the exact dtype chain reproduces the HW
error to 5 digits, and it shows 6 fp8 chunks would breach the gate
(2.06e-2), so 4 is the precision optimum.

PE roofline accounting (2.4 GHz warm): 1160 matmuls, all N=512 free
dim at 216 ns streaming => 250.6 us; measured ~268-272 us including
the ~6 us fixed framework preamble, ~3 us DMA-ramp stall in stage B,
~6 us of DoubleRow LDWEIGHTS slips (a 256-col fp8 interleaved weight
load doesn't always hide in one 216 ns matmul slot; reordering
attempts made it worse - the Tile scheduler's own interleave is near
optimal), and a ~5 us store-drain + teardown tail.

Layouts: every bulk input is host-shuffled to [128 partitions,
contiguous] so each DMA is one descriptor per partition row (issue cost
scales with descriptor count). Output is fp16 (rounding ~3e-4), widened
to f32 on host.

Schedule per core:
  stage B  q_ = relu(q @ Wq.T + bq) for all 4 b      (96 matmuls)
  per b:   qh[k, h*128+q] = q_ * h_mat on DVE
           stage A  v_ = relu(v[b] @ Wv.T + bv)      (96 fp8DR + 576 fp16)
           stage C  logits[b] = v_ @ qh + h_bias     (384 matmuls)

DMA plan: ALL bulk inputs ride one queue (sync) in exact PE-consumption
order - per-queue FIFO is the only reliable bandwidth-priority
mechanism (a second busy queue steals fair-share bandwidth from the
urgent stream, and engines bypass blocked instructions, so program
position elsewhere cannot delay a DMA). Each dma_start costs ~650ns of
issue time on the Sync engine, so the front chunks balance issue count
against completion-sem granularity. Late consumers (h_bias, v[1..3])
sit at the queue tail. Scalar issues the tiny constants and the per-vc
output stores; the last store is split per 512-col half so only 128KB
remains in flight after the final matmul. No warm-up matmuls: the
first operands land before the PE could start, and the ~3.4us
cold-clock window does real stage-B work at half rate either way.
"""

import numpy as np

B, NV, NQ = 32, 512, 128
V_DIM, Q_DIM, HK, H_OUT = 2048, 1024, 1536, 8
N_CORES = 8
BPC = B // N_CORES          # 4 batch items per core
JC = HK // 128              # 12 k-chunks
DCV = V_DIM // 128          # 16 contraction chunks for v
DC8 = 4                     # chunks of v's contraction done in fp8 DoubleRow
DCB = DCV - DC8             # 12 chunks in fp16
S8 = 8.0                    # fp8 scale split: v/8 x (8*Wv) keeps product unscaled
DCQ = Q_DIM // 128          # 8 contraction chunks for q
VC = NV // 128              # 4 v-chunks

_CACHE = {}


def _build_nc():
    import concourse.tile as tile
    from concourse import bacc, mybir
    from contextlib import ExitStack

    f16 = mybir.dt.float16
    f32 = mybir.dt.float32
    f8 = mybir.dt.float8e4

    nc = bacc.Bacc()

    # all bulk inputs are host-shuffled to [128, contiguous] so every DMA
    # is one descriptor per partition row (issue time scales with
    # descriptor count, and the DMA engines hit peak rate on 2KB+ runs)
    vT = nc.declare_dram_parameter("vT", [BPC, 128, DCB * NV], f16, isOutput=False)
    vT8 = nc.declare_dram_parameter("vT8", [BPC, 128, DC8 * NV], f8, isOutput=False)
    qT = nc.declare_dram_parameter("qT", [128, DCQ * BPC * NQ], f16, isOutput=False)
    WvT = nc.declare_dram_parameter("WvT", [128, DCB * HK], f16, isOutput=False)
    Wv8 = nc.declare_dram_parameter("Wv8", [128, JC * DC8 * 128], f8, isOutput=False)
    WqT = nc.declare_dram_parameter("WqT", [128, (JC // 4) * DCQ * 512], f16, isOutput=False)
    bvT = nc.declare_dram_parameter("bvT", [128, JC], f32, isOutput=False)
    bqT = nc.declare_dram_parameter("bqT", [128, JC], f32, isOutput=False)
    hm = nc.declare_dram_parameter("hm", [128, JC, H_OUT], f32, isOutput=False)
    hb = nc.declare_dram_parameter("hb", [128, H_OUT * NQ], f32, isOutput=False)
    out = nc.declare_dram_parameter("out", [BPC, NV, H_OUT * NQ], f16, isOutput=True)

    with ExitStack() as ctx:
        tc = ctx.enter_context(tile.TileContext(nc))
        consts = ctx.enter_context(tc.tile_pool(name="consts", bufs=1))
        qpool = ctx.enter_context(tc.tile_pool(name="qpool", bufs=1))
        vin = ctx.enter_context(tc.tile_pool(name="vin", bufs=2))
        vin8 = ctx.enter_context(tc.tile_pool(name="vin8", bufs=2))
        vact = ctx.enter_context(tc.tile_pool(name="vact", bufs=2))
        qhp = ctx.enter_context(tc.tile_pool(name="qhp", bufs=2))
        outp = ctx.enter_context(tc.tile_pool(name="outp", bufs=3))
        # 6 psAB banks: 4 in-flight per jg group + 2 spares so a group's
        # matmuls never wait on the previous group's ACTIVATE drain; psC
        # double-buffers stage C halves (warm-up tile borrows psAB).
        psAB = ctx.enter_context(tc.tile_pool(name="psAB", bufs=6, space="PSUM"))
        psC = ctx.enter_context(tc.tile_pool(name="psC", bufs=2, space="PSUM"))

        # ---- input DMAs ----------------------------------------------
        # All bulk inputs on the sync queue in EXACT consumption order:
        # bytes on the wire in the order the PE needs them. Issue cost is
        # ~0.6us each and the shared DMA-sem pool chains issues to earlier
        # completions, so a single in-order queue beats "parallel" issue.
        # Front: (qt[d], wq[jg0,d]) pairs so the first matmul only waits
        # for 2 small chunks.
        # Geometric chunk sizing: tiny first chunks (early PE start), then
        # growing chunks; all slices are contiguous in DRAM.
        qT_r = qT.rearrange("p (d n) -> p d n", d=DCQ)
        qt_sb = qpool.tile([128, DCQ, BPC * NQ], f16)
        WqT_r = WqT.rearrange("p (g d n) -> p g d n", g=JC // 4, d=DCQ)
        wq_sb = consts.tile([128, JC // 4, DCQ, 512], f16)
        # front chunking balances two costs: each dma_start takes ~650ns
        # of Sync-engine issue time (too many small chunks delay the bulk
        # stream), while too-large chunks delay the completion sem the
        # first matmuls wait on. {0},{1:4},{4:8} keeps the first matmul at
        # ~7.6us and the per-d arrival just ahead of stage B consumption.
        # The very first pair rides TWO queues (qt on sync, wq on scalar)
        # so the two transfers the first matmul waits on run concurrently
        # on separate rings - the DMA path is cold (~127GB/s) for the
        # first few us and serializing them costs ~2us. The scalar queue
        # is otherwise idle until its constants matter (~19us).
        nc.sync.dma_start(out=qt_sb[:, 0:1, :], in_=qT_r[:, 0:1, :])
        nc.scalar.dma_start(out=wq_sb[:, 0, 0:1, :], in_=WqT_r[:, 0, 0:1, :])
        nc.sync.dma_start(out=qt_sb[:, 1:4, :], in_=qT_r[:, 1:4, :])
        nc.scalar.dma_start(out=wq_sb[:, 0, 1:4, :], in_=WqT_r[:, 0, 1:4, :])
        nc.sync.dma_start(out=qt_sb[:, 4:8, :], in_=qT_r[:, 4:8, :])
        nc.sync.dma_start(out=wq_sb[:, 0, 4:8, :], in_=WqT_r[:, 0, 4:8, :])
        nc.sync.dma_start(out=wq_sb[:, 1, :, :], in_=WqT_r[:, 1, :, :])
        nc.sync.dma_start(out=wq_sb[:, 2, :, :], in_=WqT_r[:, 2, :, :])

        # scalar queue: small constants (bq needed at first ACTIVATE
        # ~19us), then per-vc output stores later.
        bq_sb = consts.tile([128, JC], f32)
        nc.scalar.dma_start(out=bq_sb, in_=bqT[:, :])
        hm_sb = consts.tile([128, JC, H_OUT], f32)
        nc.scalar.dma_start(out=hm_sb, in_=hm[:, :, :])
        bv_sb = consts.tile([128, JC], f32)
        nc.scalar.dma_start(out=bv_sb, in_=bvT[:, :])

        # WvT interleaved with b=0's vT chunks (stage A consumes
        # (wv[d], vt[d]) pairs in order), after WqT on the sync queue.
        # fp8 operands for the DoubleRow head of stage A (small, needed
        # right at stage A(b0) start)
        wv8_sb = consts.tile([128, JC, DC8, 128], f8)
        nc.sync.dma_start(
            out=wv8_sb[:, :, :, :],
            in_=Wv8.rearrange("p (j t c) -> p j t c", j=JC, t=DC8)[:, :, :, :],
        )
        vt80_sb = vin8.tile([128, DC8, NV], f8, tag="vt8", name="vt80")
        nc.sync.dma_start(
            out=vt80_sb[:, :, :],
            in_=vT8[0].rearrange("p (t n) -> p t n", t=DC8)[:, :, :],
        )
        WvT_r = WvT.rearrange("p (d j) -> p d j", d=DCB)
        wv_sb = consts.tile([128, DCB, HK], f16)
        vt0_sb = vin.tile([128, DCB, NV], f16, tag="vt", name="vt0")
        vT0_r = vT[0].rearrange("p (d n) -> p d n", d=DCB)
        for d0 in range(0, DCB, 2):
            d1 = min(d0 + 2, DCB)
            nc.sync.dma_start(out=wv_sb[:, d0:d1, :], in_=WvT_r[:, d0:d1, :])
            nc.sync.dma_start(out=vt0_sb[:, d0:d1, :], in_=vT0_r[:, d0:d1, :])
        # late consumers (hb at ~70us, v[1..3] later still) go at the TAIL
        # of the sync queue: per-queue FIFO transfer order is the only
        # reliable priority control (engines bypass blocked instructions,
        # so program position on another engine does not delay a DMA).
        hb_sb = consts.tile([128, H_OUT * NQ], f32)
        nc.sync.dma_start(out=hb_sb, in_=hb[:, :])
        vt_tiles = {0: (vt80_sb, vt0_sb)}
        for nb in range(1, BPC):
            nvt8 = vin8.tile([128, DC8, NV], f8, tag="vt8")
            nc.sync.dma_start(
                out=nvt8[:, :, :],
                in_=vT8[nb].rearrange("p (t n) -> p t n", t=DC8)[:, :, :],
            )
            nvt = vin.tile([128, DCB, NV], f16, tag="vt")
            vTn_r = vT[nb].rearrange("p (d n) -> p d n", d=DCB)
            nc.sync.dma_start(out=nvt[:, :, :], in_=vTn_r[:, :, :])
            vt_tiles[nb] = (nvt8, nvt)

        # No PE warm-up: the first stage-B operands land at ~7.4us (before
        # the PE could start anyway), and the ~3.4us cold-clock window
        # does real stage-B work at half rate either way - dummy warm-up
        # matmuls would only push stage B back.

        # ---- stage B: q_ = relu(q @ Wq.T + bq), all 4 b at once ----
        # d-outer within groups of 4 j's: weight chunk (jg, d) is consumed
        # right after its DMA lands.
        qact_sb = qpool.tile([128, JC, BPC * NQ], f16)
        for jg in range(0, JC, 4):
            pss = [psAB.tile([128, BPC * NQ], f32, tag="psAB", name=f"psB{jg}_{i}") for i in range(4)]
            for d in range(DCQ):
                for ji in range(4):
                    j = jg + ji
                    nc.tensor.matmul(
                        pss[ji],
                        lhsT=wq_sb[:, jg // 4, d, ji * 128:(ji + 1) * 128],
                        rhs=qt_sb[:, d, :],
                        start=(d == 0),
                        stop=(d == DCQ - 1),
                    )
            for ji in range(4):
                j = jg + ji
                nc.scalar.activation(
                    out=qact_sb[:, j, :],
                    in_=pss[ji],
                    func=mybir.ActivationFunctionType.Relu,
                    bias=bq_sb[:, j:j + 1],
                    scale=1.0,
                )

        for b in range(BPC):
            # ---- build Qh[b][k, h*128+q'] = q_[k, b*128+q'] * h_mat[h, k] (DVE)
            qh_sb = qhp.tile([128, JC, H_OUT * NQ], f16, tag="qh")
            for j in range(JC):
                for h in range(H_OUT):
                    nc.vector.tensor_scalar_mul(
                        qh_sb[:, j, h * NQ:(h + 1) * NQ],
                        qact_sb[:, j, b * NQ:(b + 1) * NQ],
                        hm_sb[:, j, h:h + 1],
                    )

            # ---- stage A: v_[b] = relu(v[b] @ Wv.T + bv), transposed layout
            # first DC8 contraction chunks as two fp8 DoubleRow matmuls per
            # j (half the cycles of the 4 fp16 matmuls they replace), rest
            # accumulates in fp16 on the same PSUM bank
            vt8_sb, vt_sb = vt_tiles.pop(b)
            vact_sb = vact.tile([128, JC, NV], f16, tag="vact")
            for jg in range(0, JC, 4):
                pss = [psAB.tile([128, NV], f32, tag="psAB", name=f"psA{b}_{jg}_{i}") for i in range(4)]
                for t in range(DC8 // 2):
                    for ji in range(4):
                        j = jg + ji
                        nc.tensor.matmul(
                            pss[ji],
                            lhsT=wv8_sb[:, j, 2 * t:2 * t + 2, :],
                            rhs=vt8_sb[:, 2 * t:2 * t + 2, :],
                            start=(t == 0),
                            stop=False,
                            perf_mode=mybir.MatmulPerfMode.DoubleRow,
                        )
                for d in range(DCB):
                    for ji in range(4):
                        j = jg + ji
                        nc.tensor.matmul(
                            pss[ji],
                            lhsT=wv_sb[:, d, j * 128:(j + 1) * 128],
                            rhs=vt_sb[:, d, :],
                            start=False,
                            stop=(d == DCB - 1),
                        )
                for ji in range(4):
                    j = jg + ji
                    nc.scalar.activation(
                        out=vact_sb[:, j, :],
                        in_=pss[ji],
                        func=mybir.ActivationFunctionType.Relu,
                        bias=bv_sb[:, j:j + 1],
                        scale=1.0,
                    )

            # ---- stage C: logits[b] = v_[b] @ Qh[b] (contract over k)
            # per-bank PSUM tiles: the add for half 0 runs while half 1's
            # matmuls are still accumulating.
            for vc in range(VC):
                po = [psC.tile([128, 512], f32, tag="psC", name=f"po{b}_{vc}_{i}") for i in range(2)]
                o_sb = outp.tile([128, H_OUT * NQ], f16, tag="osb")
                last = b == BPC - 1 and vc == VC - 1
                for nh in range(2):
                    for j in range(JC):
                        nc.tensor.matmul(
                            po[nh],
                            lhsT=vact_sb[:, j, vc * 128:(vc + 1) * 128],
                            rhs=qh_sb[:, j, nh * 512:(nh + 1) * 512],
                            start=(j == 0),
                            stop=(j == JC - 1),
                        )
                    sl = slice(nh * 512, (nh + 1) * 512)
                    nc.vector.tensor_add(o_sb[:, sl], po[nh], hb_sb[:, sl])
                    if last:
                        # half 0's store overlaps half 1's matmuls, so only
                        # 128KB remain in flight after the final matmul
                        nc.scalar.dma_start(
                            out=out[b, vc * 128:(vc + 1) * 128, sl],
                            in_=o_sb[:, sl],
                        )
                if not last:
                    nc.scalar.dma_start(
                        out=out[b, vc * 128:(vc + 1) * 128, :], in_=o_sb
                    )

    nc.compile()
    return nc


def kernel(v, q, Wv, bv, Wq, bq, h_mat, h_bias):
    import ml_dtypes
    from concourse import bass_utils

    f16 = np.float16

    if "nc" not in _CACHE:
        _CACHE["nc"] = _build_nc()
    nc = _CACHE["nc"]

    v = np.asarray(v, dtype=np.float32)
    q = np.asarray(q, dtype=np.float32)
    Wv = np.asarray(Wv, dtype=np.float32)
    Wq = np.asarray(Wq, dtype=np.float32)
    bv = np.asarray(bv, dtype=np.float32)
    bq = np.asarray(bq, dtype=np.float32)
    h_mat = np.asarray(h_mat, dtype=np.float32)
    h_bias = np.asarray(h_bias, dtype=np.float32)

    f8 = ml_dtypes.float8_e4m3
    D8 = DC8 * 128
    # [128, contiguous] layouts: x[p, d, ...] = orig[d*128+p, ...]
    vTf = v.transpose(0, 2, 1)                                        # (B, 2048, 512)
    vT = (
        vTf[:, D8:].reshape(B, DCB, 128, NV)
        .transpose(0, 2, 1, 3).reshape(B, 128, DCB * NV)
    ).astype(f16)                                                     # (B, 128, 6144)
    vT8 = (
        (vTf[:, :D8] / S8).reshape(B, DC8, 128, NV)
        .transpose(0, 2, 1, 3).reshape(B, 128, DC8 * NV)
    ).astype(f8)                                                      # (B, 128, 2048)
    WvT = np.ascontiguousarray(
        Wv.T[D8:].reshape(DCB, 128, HK).transpose(1, 0, 2).reshape(128, DCB * HK)
    ).astype(f16)                                                     # (128, 18432)
    # Wv8[p, j, t, c] = Wv[j*128+c, t*128+p] * S8
    Wv8 = np.ascontiguousarray(
        (Wv.T[:D8] * S8).reshape(DC8, 128, JC, 128).transpose(1, 2, 0, 3)
        .reshape(128, JC * DC8 * 128)
    ).astype(f8)                                                      # (128, 6144)
    # WqT[p, jg, d, jn] = Wq[(jg*4)*128 + jn, d*128+p]
    WqT = np.ascontiguousarray(
        Wq.T.reshape(DCQ, 128, JC // 4, 512).transpose(1, 2, 0, 3)
        .reshape(128, (JC // 4) * DCQ * 512)
    ).astype(f16)                                                     # (128, 12288)
    bvT = np.ascontiguousarray(bv.reshape(JC, 128).T)                 # (128, 12)
    bqT = np.ascontiguousarray(bq.reshape(JC, 128).T)
    # hm[p, jc, h] = h_mat[h, jc*128+p]
    hmP = np.ascontiguousarray(h_mat.reshape(H_OUT, JC, 128).transpose(2, 1, 0))
    hbB = np.ascontiguousarray(
        np.broadcast_to(np.repeat(h_bias, NQ)[None, :], (128, H_OUT * NQ))
    )

    in_maps = []
    for c in range(N_CORES):
        bs = slice(BPC * c, BPC * (c + 1))
        qTc = np.ascontiguousarray(
            q[bs].transpose(2, 0, 1).reshape(DCQ, 128, BPC * NQ)
            .transpose(1, 0, 2).reshape(128, DCQ * BPC * NQ)
        ).astype(f16)
        in_maps.append({
            "vT": vT[bs],
            "vT8": vT8[bs],
            "qT": qTc,
            "WvT": WvT,
            "Wv8": Wv8,
            "WqT": WqT,
            "bvT": bvT,
            "bqT": bqT,
            "hm": hmP,
            "hb": hbB,
        })

    res = bass_utils.run_bass_kernel_spmd(nc, in_maps, list(range(N_CORES)))
    outs = np.concatenate(
        [res.results[c]["out"].astype(np.float32) for c in range(N_CORES)], axis=0
    )
    # (32, 512, 1024) -> (32, 512, 8, 128) -> (32, 8, 512, 128)
    logits = outs.reshape(B, NV, H_OUT, NQ).transpose(0, 2, 1, 3)
    return np.ascontiguousarray(logits)
